# revision 1
# baseline (speedup 1.0000x reference)
"""ECE (expected calibration error) kernel for Trainium2, 8 NeuronCores.

Math (matches torch ECELoss(n_bins=20) / the jax reference):
    conf_i = max_c outputs[i, c]
    acc_i  = 1[outputs[i, labels_i] == conf_i]   (== argmax correct; exact on
             this data — no fp32 ties)
    S[i,b] = conf_i > edge_b for the NB+1 edges b/NB
    cum[b] = sum_i S[i,b] * v_i  for v in {conf, acc}
    sum_v[b] = cum[b] - cum[b+1]        (equal-width (lo, hi] bins + clip)
    ece = sum_b |sum_conf[b] - sum_acc[b]| / N

Key design decisions (vs a straightforward fp32 port):
  * fp16 input: halves HBM traffic (the kernel is DMA/DVE balanced at
    ~90 us/core; fp32 would be ~180 us DMA-bound) and unlocks the DVE
    2x_1P perf mode for tensor_tensor ops. End-to-end ECE error from
    fp16 quantization is ~1e-4 relative on this data (gate is 2e-2).
  * host-side column swap: x[i, labels_i] is swapped into column 0, so
    the per-sample "picked" value is a strided column read instead of a
    20-instruction gather pass (row max is permutation-invariant).
  * conf via a TT-max halving tree (128->64->...->4) at 2 elems/cycle,
    then one 1x tensor_reduce over the last 4 columns. A plain
    tensor_reduce over all 128 classes is 1x-only and ~2x slower.
  * acc = tensor_tensor(is_equal)(picked, conf) on DVE (fp16 exact).
  * S with edges pre-replicated J times in a [b, t] layout so both
    streams have unit stride (2x-eligible, fewer AP row restarts).
  * per-jumbo matmuls accumulate cum[(h,t),(b,t')] into one PSUM bank
    across the whole shard; host sums the 8 cores' partials, undoes the
    jumbo cross-product layout, does the 21->20 differencing and |.|/N.
  * GpSimd is avoided entirely: measured ~2.7 us/instruction for the
    step-matrix ops here (vs ~0.5 us on DVE).
Pad rows are zero with column 0 = -1 => conf = 0 (outside every bin,
edge_0 = 0) and picked != conf => acc = 0: they contribute nothing.

Measured ~105 us per core-shard pass vs the ~88 us fp16 HBM roofline
(65.5 MB/core fp32 input read as 32.7 MB fp16), engine-balanced:
DVE ~86 us busy, DMA ~88 us, PE ~18 us.
"""

import numpy as np

P = 128          # SBUF partitions (samples per tile)
C = 128          # classes
NB = 20          # ECE bins
NE = NB + 1      # bin edges
NCORES = 8
G = 70           # tiles per group (per DMA / per batched vector op)
J = 10           # tiles per jumbo matmul (M = 2*J <= 128, N = J*NE <= 512)
TAIL = 4         # tree switches to one tensor_reduce at this width
XBUFS = 5        # x-tile double-buffer depth


def build_nc(jr, repeat=1, unroll=1):
    """Build the Bass module for one core with JR rows per partition.

    repeat > 1 wraps the group loop in an on-device For_i that recomputes
    the same result repeat times (PSUM restarts each trip) — used only for
    perf measurement via run-time deltas. unroll unrolls the loop body to
    amortize the For_i all-engine barrier.
    """
    import concourse.bacc as bacc
    import concourse.mybir as mybir
    from concourse.tile import TileContext

    f16 = mybir.dt.float16
    f32 = mybir.dt.float32
    Alu = mybir.AluOpType
    g = G
    ng = jr // g
    assert jr % g == 0 and g % J == 0
    nj = g // J

    nc = bacc.Bacc("TRN2", target_bir_lowering=False)
    x = nc.dram_tensor("x", (P, jr, C), f16, kind="ExternalInput")
    # edges replicated J times, layout [NE, J]: value e_b at [b, :] — lets
    # the S compare keep unit stride on both streams
    consts = nc.dram_tensor("consts", (P, NE, J), f16, kind="ExternalInput")
    out = nc.dram_tensor("out", (2 * J, NE * J), f32, kind="ExternalOutput")

    with TileContext(nc) as tc:
        with (
            tc.tile_pool(name="consts", bufs=1) as cpool,
            tc.tile_pool(name="xin", bufs=XBUFS) as xpool,
            tc.tile_pool(name="tr", bufs=3) as tpool,
            tc.tile_pool(name="vt", bufs=3) as vpool,
            tc.tile_pool(name="st", bufs=3) as spool,
            tc.tile_pool(name="res", bufs=1) as rpool,
            tc.tile_pool(name="acc", bufs=1, space="PSUM") as ppool,
        ):
            constsb = cpool.tile([P, NE, J], f16)
            nc.sync.dma_start(constsb[:], consts[:])
            edgesb = constsb[:]

            psum = ppool.tile([2 * J, NE * J], f32)

            def group_body(gi):
                xt = xpool.tile([P, g, C], f16)
                nc.sync.dma_start(xt[:], x[:, gi * g:(gi + 1) * g, :])
                x3 = xt[:]

                # vt free layout: per jumbo j a contiguous [conf(J)|acc(J)]
                # block, so each matmul's stationary AP is one free dim.
                vt = vpool.tile([P, nj, 2 * J], f16)
                vt4 = vt[:].rearrange("p j (h t) -> p j h t", h=2)
                confv = vt4[:, :, 0, :]

                # max tree: fp16 TT-max runs 2 elems/cycle (2x_1P);
                # tensor_reduce is 1x-only, so halve down to TAIL wide.
                w = C
                src = x3
                while w > TAIL:
                    h = w // 2
                    dst = tpool.tile([P, g, h], f16)
                    nc.vector.tensor_tensor(
                        dst[:], src[:, :, 0:h], src[:, :, h:w], Alu.max
                    )
                    src, w = dst[:], h
                src4 = src.rearrange("p (j t) c -> p j t c", j=nj)
                nc.vector.tensor_reduce(
                    confv, src4, axis=mybir.AxisListType.X, op=Alu.max
                )

                # acc = (picked == conf); picked is column 0 (host swap)
                picked = x3.rearrange("p (j t) c -> p j t c", j=nj)[
                    :, :, :, 0:1
                ]
                nc.vector.tensor_tensor(
                    vt4[:, :, 1, :][:, :, :, None],
                    picked,
                    confv[:, :, :, None],
                    Alu.is_equal,
                )

                # S[i,b,t] = conf[i,t] > edge[b]; t innermost on both
                # streams (edges pre-replicated over t)
                st = spool.tile([P, nj, NE, J], f16)
                conf4 = confv[:, :, None, :].broadcast_to([P, nj, NE, J])
                edges4 = edgesb[:, None, :, :].broadcast_to([P, nj, NE, J])
                nc.vector.tensor_tensor(st[:], conf4, edges4, Alu.is_gt)

                # PE: accumulate cum[(h,t),(b,t')] += sum_i V[i,h,t]*S[i,b,t']
                for j in range(nj):
                    nc.tensor.matmul(
                        psum[:],
                        vt[:][:, j, :],
                        st[:][:, j, :, :],
                        start=(gi == 0 and j == 0),
                        stop=(gi == ng - 1 and j == nj - 1),
                    )

            if repeat > 1:
                trips = repeat // unroll
                assert trips * unroll == repeat
                with tc.For_i(0, trips, 1):
                    for _ in range(unroll):
                        for gi in range(ng):
                            group_body(gi)
            else:
                for gi in range(ng):
                    group_body(gi)

            res = rpool.tile([2 * J, NE * J], f32)
            nc.scalar.copy(res[:], psum[:])
            nc.sync.dma_start(out[:], res[:])

    nc.finalize()
    return nc


def _prep_inputs(outputs, labels, ncores, jr):
    """fp16 cast + swap x[i,label] into column 0 + pad/shard."""
    cap = ncores * P * jr
    n = outputs.shape[0]
    x16 = outputs.astype(np.float16)
    idx = np.arange(n)
    lab = np.asarray(labels).astype(np.int64)
    pk = x16[idx, lab].copy()
    x16[idx, lab] = x16[:, 0]
    x16[:, 0] = pk
    xpad = np.zeros((cap, C), np.float16)
    xpad[:n] = x16
    xpad[n:, 0] = -1.0  # pad rows: conf=0 (outside all bins), acc=0
    xs = xpad.reshape(ncores, P, jr, C)
    edges = (np.arange(NE, dtype=np.float32) / NB).astype(np.float16)
    edgerep = np.broadcast_to(edges[:, None], (NE, J))
    consts = np.broadcast_to(edgerep, (P, NE, J)).copy()
    return [{"x": xs[c], "consts": consts} for c in range(ncores)]


def _decode(core_outs, n):
    acc = np.zeros((2 * J, NE * J), np.float64)
    for r in core_outs:
        acc += r
    # psum column layout is [b, t'] (edges outer, jumbo-tile inner)
    cum_conf = np.zeros(NE, np.float64)
    cum_acc = np.zeros(NE, np.float64)
    for k in range(J):
        cum_conf += acc[k, k::J]
        cum_acc += acc[J + k, k::J]
    sum_conf = cum_conf[:NB] - cum_conf[1:]
    sum_acc = cum_acc[:NB] - cum_acc[1:]
    ece = np.abs(sum_conf - sum_acc).sum() / n
    return np.array([ece], dtype=np.float32)


def kernel_impl(outputs, labels, trace=False, **build_kw):
    from concourse import bass_utils

    outputs = np.ascontiguousarray(np.asarray(outputs), dtype=np.float32)
    labels = np.asarray(labels)
    n = outputs.shape[0]
    assert outputs.shape[1] == C
    jr = -(-n // (NCORES * P * G)) * G  # ceil to a multiple of G
    nc = build_nc(jr, **build_kw)
    in_maps = _prep_inputs(outputs, labels, NCORES, jr)
    res = bass_utils.run_bass_kernel_spmd(
        nc, in_maps, core_ids=list(range(NCORES)), trace=trace
    )
    ece = _decode([r["out"] for r in res.results], n)
    return ece, res


def kernel(outputs, labels):
    ece, _ = kernel_impl(outputs, labels)
    return ece

